# revision 1
# baseline (speedup 1.0000x reference)
"""DCNv2 (deformable conv + BN + ReLU) Trainium2 Bass kernel, 8-core SPMD.

Sharding: core c owns sample b=c//4, output rows [24*(c%4), 24*(c%4)+24).
Pipeline per core:
  1. offset conv (PE, bf16)  -> om[27, pos]
  2. coefficients on DVE/ACT -> bilinear weights a[pos, k, j], gather idx
  3. dma_gather of 2KB 4-corner rows from HBM table (bf16)
  4. scale+transpose+corner-sum fused on PE: S[c9,pos] += G_kj^T @ diag(a_kj)
  5. main GEMM (PE, bf16):  out[o,pos] = sum_ch W'[ch].T @ S[ch]
  6. BN stats AllReduce (8 cores), scale/shift/ReLU on ACT.
"""

import numpy as np
import ml_dtypes

BF16 = ml_dtypes.bfloat16
B, CI, CO, H, W = 2, 256, 256, 96, 96
NCORES = 8
RB = 24                      # output rows per core
NPOS = RB * W                # 2304 positions per core
PADG = 8                     # gather-table pad on each side
GRID = H + 2 * PADG          # 112
NROWS = GRID * GRID          # 12544 table rows
NTOT = float(B * H * W)      # BN count
EPS = 1e-5

KY9 = np.repeat(np.arange(3), 3).astype(np.float32)
KX9 = np.tile(np.arange(3), 3).astype(np.float32)

_CACHE = {}


def _build_program():
    import concourse.bass as bass
    from concourse import bacc, tile, mybir

    ds = bass.ds
    f32 = mybir.dt.float32
    bf16 = mybir.dt.bfloat16
    i16 = mybir.dt.int16
    Alu = mybir.AluOpType
    Act = mybir.ActivationFunctionType

    nc = bacc.Bacc("TRN2", target_bir_lowering=False, debug=False,
                   num_devices=NCORES)

    # ---- external inputs (per-core values supplied in in_maps) ----
    tab_d = nc.dram_tensor("tab", [NROWS, 1024], bf16, kind="ExternalInput")
    slab_d = nc.dram_tensor("slab", [128, 2, RB + 2, W + 2], bf16,
                            kind="ExternalInput")
    woff_d = nc.dram_tensor("woff", [128, 2, 9, 27], bf16,
                            kind="ExternalInput")
    pypx_d = nc.dram_tensor("pypx", [96, 24, 27], f32, kind="ExternalInput")
    wdcn_d = nc.dram_tensor("wdcn", [128, 18, 2, 128], bf16,
                            kind="ExternalInput")
    ident_d = nc.dram_tensor("ident", [128, 128], bf16, kind="ExternalInput")
    gb_d = nc.dram_tensor("gb", [128, 2, 3], f32, kind="ExternalInput")
    out_d = nc.dram_tensor("out", [2, 128, NPOS], f32, kind="ExternalOutput")

    with tile.TileContext(nc) as tc:
        with (
            tc.tile_pool(name="cst", bufs=1) as cst,
            tc.tile_pool(name="sb", bufs=1) as sb,
            tc.tile_pool(name="gpool", bufs=3) as gpool,
            tc.tile_pool(name="spool", bufs=2) as spool,
            tc.tile_pool(name="dpool", bufs=2) as dpool,
            tc.tile_pool(name="ps_s", bufs=2, space="PSUM") as ps_s,
            tc.tile_pool(name="ps_o", bufs=1, space="PSUM") as ps_o,
            tc.tile_pool(name="dram", bufs=1, space="DRAM") as dram,
        ):
            # ---------- load persistent tiles ----------
            slab = cst.tile([128, 2, RB + 2, W + 2], bf16)
            nc.sync.dma_start(slab[:], slab_d[:])
            woff = cst.tile([128, 2, 9, 27], bf16)
            nc.sync.dma_start(woff[:], woff_d[:])
            pypx = cst.tile([96, 24, 27], f32)
            nc.sync.dma_start(pypx[:], pypx_d[:])
            wdcn = cst.tile([128, 18, 2, 128], bf16)
            nc.sync.dma_start(wdcn[:], wdcn_d[:])
            ident = cst.tile([128, 128], bf16)
            nc.sync.dma_start(ident[:], ident_d[:])
            gb = cst.tile([128, 2, 3], f32)
            nc.sync.dma_start(gb[:], gb_d[:])

            # ---------- phase 1: offset conv, c-part [27, pos] ----------
            om_c = sb.tile([27, 6, 384], f32)
            with tc.tile_pool(name="ps_om", bufs=2, space="PSUM") as ps_om:
                for T in range(6):
                    pom = ps_om.tile([27, 384], f32)
                    first = True
                    for ct in range(2):
                        for k in range(9):
                            ky, kx = int(KY9[k]), int(KX9[k])
                            rhs = slab[:, ct, T * 4 + ky:T * 4 + ky + 4,
                                       kx:kx + 96]
                            nc.tensor.matmul(pom[:], woff[:, ct, k, :], rhs,
                                             start=first,
                                             stop=(ct == 1 and k == 8))
                            first = False
                    nc.scalar.copy(om_c[:, T, :], pom[:])

            # ---------- phase 2: transpose om to pos-part via DRAM ----------
            om_sc = dram.tile([NPOS, 27], f32)
            # src [27p, 24t, 96w] -> scratch[(t*96+w), c]
            src = om_c[:].rearrange("p a (tl w) -> p (a tl) w", w=96)
            nc.sync.dma_start(om_sc[:].rearrange("(t w) c -> c t w", t=24), src)
            om_pos = sb.tile([96, 24, 27], f32)
            nc.sync.dma_start(
                om_pos[:], om_sc[:].rearrange("(t w) c -> w t c", t=24))

            # ---------- phase 3: coefficients ----------
            opp = sb.tile([96, 24, 27], f32)
            nc.vector.tensor_tensor(opp[:], om_pos[:], pypx[:], Alu.add)
            msk = sb.tile([96, 24, 9], f32)
            nc.scalar.activation(msk[:], opp[:, :, 18:27], Act.Sigmoid)
            pys = sb.tile([96, 24, 9], f32, tag="pys")
            pxs = sb.tile([96, 24, 9], f32, tag="pxs")
            nc.vector.tensor_scalar_add(pys[:], opp[:, :, 0:9], 16.0)
            nc.vector.tensor_scalar_add(pxs[:], opp[:, :, 9:18], 16.0)
            # floor via round(x - 0.5): (x + (2^23 - 0.5)) - 2^23.
            # Exact-integer x floors one low; harmless (bilinear continuity).
            MAGIC = 8388608.0
            fy = sb.tile([96, 24, 9], f32, tag="fy")
            fx = sb.tile([96, 24, 9], f32, tag="fx")
            iyp = sb.tile([96, 24, 9], f32, tag="iyp")
            ixp = sb.tile([96, 24, 9], f32, tag="ixp")
            nc.vector.tensor_scalar(iyp[:], pys[:], MAGIC - 0.5, -MAGIC,
                                    Alu.add, Alu.add)
            nc.vector.tensor_scalar(ixp[:], pxs[:], MAGIC - 0.5, -MAGIC,
                                    Alu.add, Alu.add)
            nc.vector.tensor_tensor(fy[:], pys[:], iyp[:], Alu.subtract)
            nc.vector.tensor_tensor(fx[:], pxs[:], ixp[:], Alu.subtract)
            # clamp to grid [-8..103] -> iyp in [8, 118]
            nc.vector.tensor_scalar(iyp[:], iyp[:], 8.0, 118.0, Alu.max,
                                    Alu.min)
            nc.vector.tensor_scalar(ixp[:], ixp[:], 8.0, 118.0, Alu.max,
                                    Alu.min)
            idxf = sb.tile([96, 24, 9], f32, tag="idxf")
            nc.vector.tensor_scalar(idxf[:], iyp[:], float(GRID), -904.0,
                                    Alu.mult, Alu.add)
            nc.vector.tensor_tensor(idxf[:], idxf[:], ixp[:], Alu.add)
            idx16 = sb.tile([96, 24, 9], i16)
            nc.vector.tensor_copy(idx16[:], idxf[:])
            wy0 = sb.tile([96, 24, 9], f32, tag="wy0")
            wx0 = sb.tile([96, 24, 9], f32, tag="wx0")
            nc.vector.tensor_scalar(wy0[:], fy[:], -1.0, 1.0, Alu.mult,
                                    Alu.add)
            nc.vector.tensor_scalar(wx0[:], fx[:], -1.0, 1.0, Alu.mult,
                                    Alu.add)
            a96 = sb.tile([96, 24, 4, 9], f32)
            for j, (wy, wx) in enumerate([(wy0, wx0), (wy0, fx),
                                          (fy, wx0), (fy, fx)]):
                nc.vector.tensor_tensor(a96[:, :, j, :], wy[:], wx[:],
                                        Alu.mult)
                nc.vector.tensor_tensor(a96[:, :, j, :], a96[:, :, j, :],
                                        msk[:], Alu.mult)

            # ---------- phase 4: repack idx + a via DRAM ----------
            idx_sc = dram.tile([20736], i16)
            # stream pos: T*3456 + kc*1152 + kk*384 + tl*96 + p
            for T in range(6):
                for k in range(9):
                    src = idx16[:, T * 4:T * 4 + 4, k]
                    dst = idx_sc[ds(T * 3456 + k * 384, 384)].rearrange(
                        "(tl p) -> p tl", tl=4)
                    nc.sync.dma_start(dst, src)
            idxw = sb.tile([128, 6, 216], i16)
            nc.vector.memset(idxw[:], 0)
            nc.sync.dma_start(
                idxw[0:16, :, :].rearrange("r T s -> r (T s)"),
                idx_sc[:].rearrange("(s r) -> r s", r=16))

            a_sc = dram.tile([NPOS, 36], f32)
            nc.sync.dma_start(
                a_sc[:].rearrange("(t p) j -> p t j", t=24),
                a96[:].rearrange("p t j4 k -> p t (j4 k)"))
            a_sb = sb.tile([128, 18, 36], f32)
            nc.sync.dma_start(a_sb[:],
                              a_sc[:].rearrange("(q p) j -> p q j", q=18))

            # ---------- phases 5-7: gather, diag-scale-transpose, GEMM ----
            out_sb = sb.tile([128, 2, NPOS], f32)
            for T in range(6):
                gt = []
                for kc in range(3):
                    g = gpool.tile([128, 9, 1024], bf16, tag="g")
                    nc.gpsimd.dma_gather(
                        g[:], tab_d[:], idxw[:, T, kc * 72:(kc + 1) * 72],
                        num_idxs=1152, num_idxs_reg=1152, elem_size=1024)
                    gt.append(g)
                s_sb = spool.tile([128, 18, 384], bf16, tag="s")
                for q in range(3):
                    qg = T * 3 + q
                    dg = dpool.tile([128, 36, 128], bf16, tag="diag")
                    for kj in range(36):
                        nc.vector.tensor_scalar_mul(
                            dg[:, kj, :], ident[:],
                            a_sb[:, qg, kj:kj + 1])
                    for third in range(3):
                        pss = ps_s.tile([128, 6, 128], f32, tag="pss")
                        for chl in range(6):
                            ch = third * 6 + chl
                            k, cfh = ch // 2, ch % 2
                            g = gt[k // 3]
                            slot = (k % 3) * 3 + q
                            for j in range(4):
                                lhsT = g[:, slot, j * 256 + cfh * 128:
                                         j * 256 + cfh * 128 + 128]
                                nc.tensor.matmul(pss[:, chl, :], lhsT,
                                                 dg[:, (j * 9 + k), :],
                                                 start=(j == 0),
                                                 stop=(j == 3))
                        nc.scalar.copy(
                            s_sb[:, third * 6:third * 6 + 6,
                                 q * 128:(q + 1) * 128], pss[:])
                for o2 in range(2):
                    po = ps_o.tile([128, 384], f32, tag="po")
                    for ch in range(18):
                        nc.tensor.matmul(po[:], wdcn[:, ch, o2, :],
                                         s_sb[:, ch, :], start=(ch == 0),
                                         stop=(ch == 17))
                    nc.vector.tensor_scalar_add(
                        out_sb[:, o2, T * 384:(T + 1) * 384], po[:],
                        gb[:, o2, 2:3])

            # ---------- phase 8: BN stats + allreduce + finish ----------
            part = sb.tile([128, 4], f32)
            scrap = sb.tile([128, NPOS], bf16)
            for o2 in range(2):
                nc.vector.tensor_reduce(part[:, 2 * o2:2 * o2 + 1],
                                        out_sb[:, o2, :],
                                        mybir.AxisListType.X, Alu.add)
                nc.scalar.activation(scrap[:], out_sb[:, o2, :], Act.Square,
                                     accum_out=part[:, 2 * o2 + 1:2 * o2 + 2])
            bin_d = dram.tile([128, 4], f32)
            bout_d = dram.tile([128, 4], f32, addr_space="Shared")
            import os as _os
            nc.gpsimd.dma_start(bin_d[:], part[:])
            if _os.environ.get("NOCC", "0") == "1":
                nc.gpsimd.dma_start(bout_d[:], bin_d[:])
            else:
                nc.gpsimd.collective_compute(
                    "AllReduce", mybir.AluOpType.add,
                    replica_groups=[list(range(NCORES))],
                    ins=[bin_d[:].opt()], outs=[bout_d[:].opt()])
            stats = sb.tile([128, 4], f32)
            nc.sync.dma_start(stats[:], bout_d[:])
            tmp = sb.tile([128, 8], f32)
            outf = sb.tile([128, NPOS], f32)
            for o2 in range(2):
                mean = tmp[:, 4 * o2 + 0:4 * o2 + 1]
                var = tmp[:, 4 * o2 + 1:4 * o2 + 2]
                s_ = tmp[:, 4 * o2 + 2:4 * o2 + 3]
                t_ = tmp[:, 4 * o2 + 3:4 * o2 + 4]
                nc.vector.tensor_scalar_mul(mean, stats[:, 2 * o2:2 * o2 + 1],
                                            1.0 / NTOT)
                nc.vector.tensor_scalar_mul(var,
                                            stats[:, 2 * o2 + 1:2 * o2 + 2],
                                            1.0 / NTOT)
                nc.vector.tensor_tensor(s_, mean, mean, Alu.mult)
                nc.vector.tensor_tensor(var, var, s_, Alu.subtract)
                nc.vector.tensor_scalar_add(var, var, EPS)
                nc.scalar.sqrt(s_, var)
                nc.vector.reciprocal(s_, s_)
                nc.vector.tensor_tensor(s_, s_, gb[:, o2, 0:1], Alu.mult)
                nc.vector.tensor_tensor(t_, mean, s_, Alu.mult)
                nc.vector.tensor_scalar_mul(t_, t_, -1.0)
                nc.vector.tensor_tensor(t_, t_, gb[:, o2, 1:2], Alu.add)
                nc.scalar.activation(outf[:], out_sb[:, o2, :], Act.Relu,
                                     bias=t_, scale=s_)
                nc.sync.dma_start(out_d[o2], outf[:])

    nc.compile()
    return nc


def _prep_inputs(x, w_off, b_off, w_dcn, b_dcn, gamma, beta):
    """Build the 8 per-core input maps (host-side sharding/layout only)."""
    x = np.asarray(x, np.float32)
    w_off = np.asarray(w_off, np.float32)
    b_off = np.asarray(b_off, np.float32)
    w_dcn = np.asarray(w_dcn, np.float32)
    b_dcn = np.asarray(b_dcn, np.float32)
    gamma = np.asarray(gamma, np.float32)
    beta = np.asarray(beta, np.float32)

    # 4-corner gather tables per sample
    P = PADG
    xp = np.zeros((B, CI, GRID + 1, GRID + 1), np.float32)
    xp[:, :, P:P + H, P:P + W] = x
    xp = xp.astype(BF16)
    tabs = []
    for b in range(B):
        t = np.empty((GRID, GRID, 4, CI), BF16)
        for j, (dy2, dx2) in enumerate([(0, 0), (0, 1), (1, 0), (1, 1)]):
            t[:, :, j, :] = np.moveaxis(
                xp[b, :, dy2:dy2 + GRID, dx2:dx2 + GRID], 0, -1)
        tabs.append(np.ascontiguousarray(t.reshape(NROWS, 1024)))

    # conv slab (1-pixel zero pad) per sample, bf16, [128, ct, 26, 98]
    xs = np.zeros((B, CI, H + 2, W + 2), np.float32)
    xs[:, :, 1:H + 1, 1:W + 1] = x
    xs = xs.astype(BF16)

    # offset-conv weights, output channels permuted to [dy*9, dx*9, m*9]
    perm = np.concatenate([np.arange(0, 17, 2), np.arange(1, 18, 2),
                           np.arange(18, 27)])
    wofp = w_off[perm]            # [27, CI, 3, 3]
    boffp = b_off[perm]
    woff_h = np.ascontiguousarray(
        wofp.reshape(27, 2, 128, 3, 3).transpose(2, 1, 3, 4, 0)
        .reshape(128, 2, 9, 27)).astype(BF16)

    # wdcn lhsT chunks: [p, ch=(k*2+cf), o2, oc] = w_dcn[o2*128+oc, cf*128+p, k]
    wd = w_dcn.reshape(CO, CI, 9)
    wdcn_h = np.ascontiguousarray(
        wd.reshape(2, 128, 2, 128, 9).transpose(3, 4, 2, 0, 1)
        .reshape(128, 9, 2, 2, 128).transpose(0, 1, 2, 3, 4)
        .reshape(128, 18, 2, 128)).astype(BF16)

    ident_h = np.eye(128, dtype=BF16)
    gb_h = np.zeros((128, 2, 3), np.float32)
    for o2 in range(2):
        gb_h[:, o2, 0] = gamma[o2 * 128:(o2 + 1) * 128]
        gb_h[:, o2, 1] = beta[o2 * 128:(o2 + 1) * 128]
        gb_h[:, o2, 2] = b_dcn[o2 * 128:(o2 + 1) * 128]

    in_maps = []
    for c in range(NCORES):
        b, rb = c // 4, c % 4
        slab_h = np.ascontiguousarray(
            xs[b].reshape(2, 128, H + 2, W + 2)
            .transpose(1, 0, 2, 3)[:, :, rb * RB:rb * RB + RB + 2, :])
        pypx_h = np.zeros((96, 24, 27), np.float32)
        pp = np.arange(96, dtype=np.float32)
        tt = np.arange(24, dtype=np.float32)
        pypx_h[:, :, 0:9] = (rb * RB - 1.0 + tt[None, :, None]
                             + KY9[None, None, :] + boffp[None, None, 0:9])
        pypx_h[:, :, 9:18] = (pp[:, None, None] - 1.0
                              + KX9[None, None, :] + boffp[None, None, 9:18])
        pypx_h[:, :, 18:27] = boffp[None, None, 18:27]
        in_maps.append({
            "tab": tabs[b], "slab": slab_h, "woff": woff_h,
            "pypx": pypx_h, "wdcn": wdcn_h, "ident": ident_h, "gb": gb_h,
        })
    return in_maps


def kernel(x, w_off, b_off, w_dcn, b_dcn, gamma, beta, _trace=False):
    from concourse.bass_utils import run_bass_kernel_spmd

    if "nc" not in _CACHE:
        _CACHE["nc"] = _build_program()
    nc = _CACHE["nc"]
    in_maps = _prep_inputs(x, w_off, b_off, w_dcn, b_dcn, gamma, beta)
    results = None
    try:
        try:
            res = run_bass_kernel_spmd(nc, in_maps,
                                       core_ids=list(range(NCORES)),
                                       trace=_trace)
        except ModuleNotFoundError:
            res = run_bass_kernel_spmd(nc, in_maps,
                                       core_ids=list(range(NCORES)),
                                       trace=False)
        _CACHE["last"] = res
        results = res.results
    except Exception:
        # hardware path unavailable: fall back to the multi-core simulator
        from concourse import bass_interp
        sim = bass_interp.MultiCoreSim(nc, NCORES)
        for c in range(NCORES):
            for name, val in in_maps[c].items():
                sim.cores[c].tensor(name)[:] = val
        sim.simulate()
        results = [{"out": np.asarray(sim.cores[c].tensor("out"))}
                   for c in range(NCORES)]
    out = np.empty((B, CO, H, W), np.float32)
    for c in range(NCORES):
        b, rb = c // 4, c % 4
        o = results[c]["out"]  # [2, 128, NPOS]
        out[b, :, rb * RB:(rb + 1) * RB, :] = o.reshape(CO, RB, W)
    return out



# revision 19
# speedup vs baseline: 3.6014x; 3.6014x over previous
"""DCNv2 (deformable conv + BN + ReLU) Trainium2 Bass kernel, 8-core SPMD.

v2: fully pipelined per T-tile (4 output rows each). Core c owns sample
b=c//4, output rows [24*(c%4), 24*(c%4)+24).

Position relabeling: within a T-tile, conv column col = t*96+w is assigned
pipeline position l = q*128 + pp*16 + r where col = r*24 + q*8 + pp.
This makes the gather-index repack DMA contiguous in 48B runs:
  idxG[r, k*24 + c] = idx16[k, r*24 + c]   (c = col%24)
and the gather consumes idxG[16, 216] in n = s*16+r order with
n = ((k%3)*3+q)*128 + (pp*16+r), exactly the corner-matmul layout.

Per T (program order; pools give cross-T overlap):
  conv(T) on PE -> coeffs(T) on DVE/ACT (conv layout [27, 384]) ->
  a-transpose on PE (3x [36,128]->[128,36]) -> idx DMA roundtrip ->
  3x dma_gather -> dg = ident*a (1 broadcast DVE op per q) ->
  corner matmuls (216) -> main GEMM (36) with BN sums via accum_out.
Tail: AllReduce of BN stats, scale/shift/ReLU, chunked stores.
"""

import numpy as np
import ml_dtypes

BF16 = ml_dtypes.bfloat16
B, CI, CO, H, W = 2, 256, 256, 96, 96
NCORES = 8
RB = 24                      # output rows per core
NPOS = RB * W                # 2304 positions per core
PADG = 8                     # gather-table pad on each side
GRID = H + 2 * PADG          # 112
NROWS = GRID * GRID          # 12544 table rows
NTOT = float(B * H * W)      # BN count
EPS = 1e-5
MAGIC = 8388608.0            # 2^23 for round-to-floor trick

KY9 = np.repeat(np.arange(3), 3).astype(np.float32)
KX9 = np.tile(np.arange(3), 3).astype(np.float32)

_CACHE = {}


def _build_program():
    import concourse.bass as bass
    from concourse import bacc, tile, mybir

    ds = bass.ds
    f32 = mybir.dt.float32
    bf16 = mybir.dt.bfloat16
    i16 = mybir.dt.int16
    Alu = mybir.AluOpType
    Act = mybir.ActivationFunctionType

    nc = bacc.Bacc("TRN2", target_bir_lowering=False, debug=False,
                   num_devices=NCORES, dynamic_dma_scratch_size=32768)

    tab_d = nc.dram_tensor("tab", [NROWS, 1024], bf16, kind="ExternalInput")
    slab_d = nc.dram_tensor("slab", [128, 2, RB + 2, W + 2], bf16,
                            kind="ExternalInput")
    woff_d = nc.dram_tensor("woff", [128, 2, 9, 96], bf16,
                            kind="ExternalInput")
    pypx_d = nc.dram_tensor("pypx", [96, 6, 384], f32, kind="ExternalInput")
    wdcn_d = nc.dram_tensor("wdcn", [128, 18, 2, 128], bf16,
                            kind="ExternalInput")
    ident_d = nc.dram_tensor("ident", [128, 128], bf16, kind="ExternalInput")
    identf_d = nc.dram_tensor("identf", [128, 128], f32, kind="ExternalInput")
    gb_d = nc.dram_tensor("gb", [128, 2, 3], f32, kind="ExternalInput")
    out_d = nc.dram_tensor("out", [2, 128, NPOS], f32, kind="ExternalOutput")

    with tile.TileContext(nc) as tc:
        with (
            tc.tile_pool(name="cst", bufs=1) as cst,
            tc.tile_pool(name="sb", bufs=1) as sb,
            tc.tile_pool(name="cf", bufs=2) as cf,
            tc.tile_pool(name="gpool", bufs=3) as gpool,
            tc.tile_pool(name="apool", bufs=2) as apool,
            tc.tile_pool(name="dpool", bufs=7) as dpool,
            tc.tile_pool(name="spool", bufs=2) as spool,
            tc.tile_pool(name="opool", bufs=3) as opool,
            tc.tile_pool(name="ps_om", bufs=1, space="PSUM") as ps_om,
            tc.tile_pool(name="ps_t", bufs=1, space="PSUM") as ps_t,
            tc.tile_pool(name="ps_s", bufs=2, space="PSUM") as ps_s,
            tc.tile_pool(name="ps_o", bufs=2, space="PSUM") as ps_o,
            tc.tile_pool(name="dram", bufs=1, space="DRAM") as dram,
        ):
            # ---------- PE warm-up: ramp p-state during input loads ----
            wident = cst.tile([128, 128], bf16)
            nc.vector.memset(wident[:], 0)
            wps = ps_om.tile([96, 384], f32, tag="pom")
            for _ in range(40):
                nc.tensor.matmul(wps[:, 0:128], wident[:, 0:96],
                                 wident[:])

            # ---------- persistent tiles ----------
            slab = cst.tile([128, 2, RB + 2, W + 2], bf16)
            nc.sync.dma_start(slab[:], slab_d[:])
            woff = cst.tile([128, 2, 9, 96], bf16)
            nc.sync.dma_start(woff[:], woff_d[:])
            pypx = cst.tile([96, 6, 384], f32)
            nc.sync.dma_start(pypx[:], pypx_d[:])
            wdcn = cst.tile([128, 18, 2, 128], bf16)
            nc.sync.dma_start(wdcn[:], wdcn_d[:])
            ident = cst.tile([128, 128], bf16)
            nc.sync.dma_start(ident[:], ident_d[:])
            identf = cst.tile([128, 128], f32)
            nc.sync.dma_start(identf[:], identf_d[:])
            gb = cst.tile([128, 2, 3], f32)
            nc.sync.dma_start(gb[:], gb_d[:])

            idxG = sb.tile([128, 2, 216], i16)
            nc.vector.memset(idxG[:], 0)
            d4 = dram.tile([2, 3456], i16)
            out_sb = sb.tile([128, 2, NPOS], f32)
            SU = sb.tile([128, 2, 18], f32)  # per-(T,q) BN sums
            SQ = sb.tile([128, 2, 6], f32)   # per-T BN sum-of-squares

            ident_b = ident[:].rearrange("p (one n) -> p one n", one=1) \
                .broadcast_to([128, 36, 128])

            def conv(T):
                pom = ps_om.tile([96, 384], f32, tag="pom")
                first = True
                for ct in range(2):
                    for k in range(9):
                        ky, kx = int(KY9[k]), int(KX9[k])
                        rhs = slab[:, ct, T * 4 + ky:T * 4 + ky + 4,
                                   kx:kx + 96]
                        nc.tensor.matmul(pom[:], woff[:, ct, k, :], rhs,
                                         start=first,
                                         stop=(ct == 1 and k == 8))
                        first = False
                return pom

            def idx_coeffs(T, pom):
                # coefficients in conv layout [<=36 part, 384 col]
                opp = cf.tile([96, 384], f32, tag="opp")
                nc.vector.tensor_tensor(opp[:], pom[:], pypx[:, T], Alu.add)
                msk = cf.tile([9, 384], f32, tag="msk")
                nc.scalar.activation(msk[:], opp[64:73], Act.Sigmoid)
                iyx = cf.tile([64, 384], f32, tag="iyx")
                # floor via round(x - 0.5); exact-int x floors one low
                # (harmless by bilinear continuity). y rows 0:9, x rows
                # 32:41; in-between rows are well-defined junk.
                nc.vector.tensor_scalar(iyx[:], opp[0:64], MAGIC - 0.5,
                                        -MAGIC, Alu.add, Alu.add)
                fyx = cf.tile([64, 384], f32, tag="fyx")
                nc.vector.tensor_tensor(fyx[:], opp[0:64], iyx[:],
                                        Alu.subtract)
                nc.vector.tensor_scalar(iyx[:], iyx[:], 8.0, 118.0, Alu.max,
                                        Alu.min)
                idxf = cf.tile([9, 384], f32, tag="idxf")
                nc.vector.tensor_scalar(idxf[:], iyx[0:9], float(GRID),
                                        -904.0, Alu.mult, Alu.add)
                nc.vector.tensor_tensor(idxf[:], idxf[:], iyx[32:41],
                                        Alu.add)
                idx16 = cf.tile([9, 384], i16, tag="idx16")
                nc.vector.tensor_copy(idx16[:], idxf[:])

                # idx repack via DRAM (contiguous 48B runs), then gathers
                slot = T % 2
                nc.sync.dma_start(
                    d4[slot].rearrange("(r k c) -> k r c", r=16, k=9),
                    idx16[:].rearrange("k (r c) -> k r c", r=16))
                nc.sync.dma_start(idxG[0:16, slot, :],
                                  d4[slot].rearrange("(r s) -> r s", r=16))
                gt = []
                for kc in range(3):
                    g = gpool.tile([128, 9, 1024], bf16, tag="g")
                    nc.gpsimd.dma_gather(
                        g[:], tab_d[:], idxG[:, slot, kc * 72:(kc + 1) * 72],
                        num_idxs=1152, num_idxs_reg=1152, elem_size=1024)
                    gt.append(g)
                return msk, fyx, gt

            def a_coeffs(T, fyx):
                # j-corner products at partition groups 0/32/64/96; the
                # full 32-row ops also fill filler rows with finite junk
                wyx0 = cf.tile([64, 384], f32, tag="wyx0")
                nc.gpsimd.tensor_scalar(wyx0[:], fyx[:], -1.0, 1.0, Alu.mult,
                                        Alu.add)
                aFm = cf.tile([128, 384], f32, tag="aFm")
                nc.gpsimd.tensor_tensor(aFm[0:32], wyx0[0:32], wyx0[32:64],
                                        Alu.mult)
                nc.gpsimd.tensor_tensor(aFm[32:64], wyx0[0:32], fyx[32:64],
                                        Alu.mult)
                nc.gpsimd.tensor_tensor(aFm[64:96], fyx[0:32], wyx0[32:64],
                                        Alu.mult)
                nc.gpsimd.tensor_tensor(aFm[96:128], fyx[0:32], fyx[32:64],
                                        Alu.mult)
                return aFm

            def a_transpose(T, aFm, msk):
                # PE transposes per q-block: aFm [128,(pp,r)] -> [128,128]
                # (j-groups at cols 0/32/64/96), mask [9,..] -> cols 128:137
                a_ps = ps_t.tile([128, 3, 137], f32, tag="aps")
                aF_v = aFm[:].rearrange("k (r q2 pp) -> k q2 pp r",
                                        r=16, q2=3)
                m_v = msk[:].rearrange("k (r q2 pp) -> k q2 pp r",
                                       r=16, q2=3)
                for q in range(3):
                    nc.tensor.matmul(a_ps[:, q, 0:128], aF_v[:, q],
                                     identf[:], is_transpose=True)
                    nc.tensor.matmul(a_ps[:, q, 128:137], m_v[:, q],
                                     identf[0:9, 0:9], is_transpose=True)
                a_pos = apool.tile([128, 3, 137], f32, tag="apos")
                nc.vector.tensor_copy(a_pos[:], a_ps[:])
                return a_pos

            def dg_one(T, t, q, a_pos):
                # 12 diags for (third t, q-block): k in {3t..3t+2} x 4 corners
                dgs = dpool.tile([128, 12, 128], bf16, tag="dg")
                for kk in range(3):
                    k = t * 3 + kk
                    for j in range(4):
                        col = j * 32 + k
                        nc.vector.tensor_scalar(
                            dgs[:, kk * 4 + j, :], ident[:],
                            a_pos[:, q, col:col + 1],
                            a_pos[:, q, 128 + k:129 + k],
                            Alu.mult, Alu.mult)
                return dgs

            def corner_tq(T, t, q, g, dgs, s_sb):
                # third t only reads gather tile t
                pss = ps_s.tile([128, 6, 128], f32, tag="pss")
                for chl in range(6):
                    k, cfh = t * 3 + chl // 2, chl % 2
                    slot9 = (k % 3) * 3 + q
                    for j in range(4):
                        lhsT = g[:, slot9, j * 256 + cfh * 128:
                                 j * 256 + cfh * 128 + 128]
                        nc.tensor.matmul(pss[:, chl, :], lhsT,
                                         dgs[:, (chl // 2) * 4 + j, :],
                                         start=(j == 0), stop=(j == 3))
                nc.scalar.copy(s_sb[:, t * 6:t * 6 + 6,
                                    q * 128:(q + 1) * 128], pss[:])

            def gemm_q(T, q, s_sb):
                po = ps_o.tile([128, 2, 128], f32, tag="po")
                for o2 in range(2):
                    for ch in range(18):
                        nc.tensor.matmul(po[:, o2, :], wdcn[:, ch, o2, :],
                                         s_sb[:, ch, q * 128:(q + 1) * 128],
                                         start=(ch == 0), stop=(ch == 17))
                for o2 in range(2):
                    osl = out_sb[:, o2, T * 384 + q * 128:
                                 T * 384 + (q + 1) * 128]
                    nc.scalar.activation(osl, po[:, o2, :], Act.Identity,
                                         bias=gb[:, o2, 2:3],
                                         accum_out=SU[:, o2,
                                                      T * 3 + q:T * 3 + q + 1])

            def square(T):
                for o2 in range(2):
                    scrap = opool.tile([128, 384], bf16, tag="scrap")
                    nc.scalar.activation(scrap[:],
                                         out_sb[:, o2,
                                                T * 384:(T + 1) * 384],
                                         Act.Square,
                                         accum_out=SQ[:, o2, T:T + 1])

            # ---------- software-pipelined main loop ----------
            # corner loops are third-major: third t consumes only gather
            # tile t, so compute starts as soon as the first gather lands
            pom = conv(0)
            msk, fyx, gt = idx_coeffs(0, pom)
            aF = a_coeffs(0, fyx)
            a_pos = a_transpose(0, aF, msk)
            junk = sb.tile([1, 2], f32)
            for T in range(6):
                if T == 5:
                    # preload sqrt act-table after the last Sigmoid (the
                    # Square dep pins it late; Relu/Copy/Square are in the
                    # sqrt set too, so the tail needs no further switch)
                    nc.scalar.activation(junk[:, 0:1], SU[0:1, 1, 14:15],
                                         Act.Square)
                    nc.scalar.sqrt(junk[:, 1:2], junk[:, 0:1])
                s_sb = spool.tile([128, 18, 384], bf16, tag="s")
                d0 = [dg_one(T, 0, q, a_pos) for q in range(3)]
                d1 = [dg_one(T, 1, q, a_pos) for q in range(3)]
                for q in range(3):
                    corner_tq(T, 0, q, gt[0], d0[q], s_sb)
                if T < 5:
                    pom = conv(T + 1)
                    msk, fyx, gt_n = idx_coeffs(T + 1, pom)
                for q in range(3):
                    corner_tq(T, 1, q, gt[1], d1[q], s_sb)
                if T < 5:
                    aF = a_coeffs(T + 1, fyx)
                d2 = [dg_one(T, 2, 0, a_pos), dg_one(T, 2, 1, a_pos), None]
                corner_tq(T, 2, 0, gt[2], d2[0], s_sb)
                if T < 5:
                    a_posn = a_transpose(T + 1, aF, msk)
                gemm_q(T, 0, s_sb)
                d2[2] = dg_one(T, 2, 2, a_pos)
                corner_tq(T, 2, 1, gt[2], d2[1], s_sb)
                gemm_q(T, 1, s_sb)
                corner_tq(T, 2, 2, gt[2], d2[2], s_sb)
                gemm_q(T, 2, s_sb)
                square(T)
                if T < 5:
                    gt = gt_n
                    a_pos = a_posn

            # ---------- BN stats + allreduce + finish ----------
            part = sb.tile([128, 4], f32)
            for o2 in range(2):
                nc.vector.tensor_reduce(part[:, 2 * o2:2 * o2 + 1],
                                        SU[:, o2, :],
                                        mybir.AxisListType.X, Alu.add)
                nc.vector.tensor_reduce(part[:, 2 * o2 + 1:2 * o2 + 2],
                                        SQ[:, o2, :],
                                        mybir.AxisListType.X, Alu.add)
            bin_d = dram.tile([128, 4], f32)
            bout_d = dram.tile([128, 4], f32, addr_space="Shared")
            import os as _os
            nc.sync.dma_start(bin_d[:], part[:])
            if _os.environ.get("NOCC", "0") == "1":
                nc.gpsimd.dma_start(bout_d[:], bin_d[:])
            else:
                nc.gpsimd.collective_compute(
                    "AllReduce", mybir.AluOpType.add,
                    replica_groups=[list(range(NCORES))],
                    ins=[bin_d[:].opt()], outs=[bout_d[:].opt()])
            stats = sb.tile([128, 4], f32)
            nc.sync.dma_start(stats[:], bout_d[:])
            tmp = sb.tile([128, 8], f32)
            for o2 in range(2):
                mean = tmp[:, 4 * o2 + 0:4 * o2 + 1]
                var = tmp[:, 4 * o2 + 1:4 * o2 + 2]
                s_ = tmp[:, 4 * o2 + 2:4 * o2 + 3]
                t_ = tmp[:, 4 * o2 + 3:4 * o2 + 4]
                nc.vector.tensor_scalar_mul(mean, stats[:, 2 * o2:2 * o2 + 1],
                                            1.0 / NTOT)
                nc.vector.tensor_scalar_mul(var,
                                            stats[:, 2 * o2 + 1:2 * o2 + 2],
                                            1.0 / NTOT)
                nc.vector.tensor_tensor(s_, mean, mean, Alu.mult)
                nc.vector.tensor_tensor(var, var, s_, Alu.subtract)
                nc.vector.tensor_scalar_add(var, var, EPS)
                nc.scalar.sqrt(s_, var)
                nc.vector.reciprocal(s_, s_)
                nc.vector.tensor_tensor(s_, s_, gb[:, o2, 0:1], Alu.mult)
                nc.vector.tensor_tensor(t_, mean, s_, Alu.mult)
                nc.vector.tensor_scalar_mul(t_, t_, -1.0)
                nc.vector.tensor_tensor(t_, t_, gb[:, o2, 1:2], Alu.add)
            for o2 in range(2):
                s_ = tmp[:, 4 * o2 + 2:4 * o2 + 3]
                t_ = tmp[:, 4 * o2 + 3:4 * o2 + 4]
                for hh in range(3):
                    outf = opool.tile([128, 768], f32, tag="outf")
                    nc.scalar.activation(
                        outf[:], out_sb[:, o2, hh * 768:(hh + 1) * 768],
                        Act.Relu, bias=t_, scale=s_)
                    nc.sync.dma_start(out_d[o2, :, hh * 768:(hh + 1) * 768],
                                      outf[:])

    nc.compile()
    return nc


# position permutation: l = q*128 + pp*16 + r for col = r*24 + q*8 + pp
_COL = np.arange(384)
_LUT = (_COL % 24 // 8) * 128 + (_COL % 8) * 16 + _COL // 24  # col -> l


def _prep_inputs(x, w_off, b_off, w_dcn, b_dcn, gamma, beta):
    """Build the 8 per-core input maps (host-side sharding/layout only)."""
    x = np.asarray(x, np.float32)
    w_off = np.asarray(w_off, np.float32)
    b_off = np.asarray(b_off, np.float32)
    w_dcn = np.asarray(w_dcn, np.float32)
    b_dcn = np.asarray(b_dcn, np.float32)
    gamma = np.asarray(gamma, np.float32)
    beta = np.asarray(beta, np.float32)

    # 4-corner gather tables per sample
    P = PADG
    xp = np.zeros((B, CI, GRID + 1, GRID + 1), np.float32)
    xp[:, :, P:P + H, P:P + W] = x
    xp = xp.astype(BF16)
    tabs = []
    for b in range(B):
        t = np.empty((GRID, GRID, 4, CI), BF16)
        for j, (dy2, dx2) in enumerate([(0, 0), (0, 1), (1, 0), (1, 1)]):
            t[:, :, j, :] = np.moveaxis(
                xp[b, :, dy2:dy2 + GRID, dx2:dx2 + GRID], 0, -1)
        tabs.append(np.ascontiguousarray(t.reshape(NROWS, 1024)))

    # conv slab (1-pixel zero pad) per sample, bf16, [128, ct, 26, 98]
    xs = np.zeros((B, CI, H + 2, W + 2), np.float32)
    xs[:, :, 1:H + 1, 1:W + 1] = x
    xs = xs.astype(BF16)

    # offset-conv weights, output channels permuted to [dy*9, dx*9, m*9]
    perm = np.concatenate([np.arange(0, 17, 2), np.arange(1, 18, 2),
                           np.arange(18, 27)])
    wofp = w_off[perm]            # [27, CI, 3, 3]
    boffp = b_off[perm]
    w27 = np.ascontiguousarray(
        wofp.reshape(27, 2, 128, 3, 3).transpose(2, 1, 3, 4, 0)
        .reshape(128, 2, 9, 27)).astype(BF16)
    # out channels embedded at partition groups 0:9 (y), 32:41 (x),
    # 64:73 (m) so engine slices start at multiples of 32
    woff_h = np.zeros((128, 2, 9, 96), BF16)
    woff_h[:, :, :, 0:9] = w27[:, :, :, 0:9]
    woff_h[:, :, :, 32:41] = w27[:, :, :, 9:18]
    woff_h[:, :, :, 64:73] = w27[:, :, :, 18:27]

    # wdcn lhsT chunks: [p, ch=(k*2+cf), o2, oc] = w_dcn[o2*128+oc, cf*128+p, k]
    wd = w_dcn.reshape(CO, CI, 9)
    wdcn_h = np.ascontiguousarray(
        wd.reshape(2, 128, 2, 128, 9).transpose(3, 4, 2, 0, 1)
        .reshape(128, 9, 2, 2, 128).transpose(0, 1, 2, 3, 4)
        .reshape(128, 18, 2, 128)).astype(BF16)

    ident_h = np.eye(128, dtype=BF16)
    identf_h = np.eye(128, dtype=np.float32)
    gb_h = np.zeros((128, 2, 3), np.float32)
    for o2 in range(2):
        gb_h[:, o2, 0] = gamma[o2 * 128:(o2 + 1) * 128]
        gb_h[:, o2, 1] = beta[o2 * 128:(o2 + 1) * 128]
        gb_h[:, o2, 2] = b_dcn[o2 * 128:(o2 + 1) * 128]

    tt = np.arange(4, dtype=np.float32)   # row within T-tile
    ww = np.arange(96, dtype=np.float32)
    in_maps = []
    for c in range(NCORES):
        b, rb = c // 4, c % 4
        slab_h = np.ascontiguousarray(
            xs[b].reshape(2, 128, H + 2, W + 2)
            .transpose(1, 0, 2, 3)[:, :, rb * RB:rb * RB + RB + 2, :])
        # pypx [96, 6T, 384col]: +16 (grid offset) folded into y/x rows;
        # rows 0:9 = y base, 32:41 = x base, 64:73 = mask bias
        pypx_h = np.zeros((96, 6, 384), np.float32)
        for T in range(6):
            py = np.broadcast_to(
                rb * RB + T * 4 - 1.0 + 16.0 + tt[None, :, None]
                + KY9[:, None, None] + boffp[0:9, None, None], (9, 4, 96))
            px = (ww[None, None, :] - 1.0 + 16.0
                  + KX9[:, None, None] + boffp[9:18, None, None])
            px = np.broadcast_to(px, (9, 4, 96))
            pypx_h[0:9, T] = py.reshape(9, 384)
            pypx_h[32:41, T] = px.reshape(9, 384)
            pypx_h[64:73, T] = boffp[18:27, None]
        in_maps.append({
            "tab": tabs[b], "slab": slab_h, "woff": woff_h,
            "pypx": pypx_h, "wdcn": wdcn_h, "ident": ident_h,
            "identf": identf_h, "gb": gb_h,
        })
    return in_maps


def kernel(x, w_off, b_off, w_dcn, b_dcn, gamma, beta, _trace=False):
    import os
    if "nc" not in _CACHE:
        _CACHE["nc"] = _build_program()
    nc = _CACHE["nc"]
    in_maps = _prep_inputs(x, w_off, b_off, w_dcn, b_dcn, gamma, beta)
    results = None
    if os.environ.get("FORCE_SIM", "0") == "1":
        from concourse import bass_interp
        sim = bass_interp.MultiCoreSim(nc, NCORES)
        for c in range(NCORES):
            for name, val in in_maps[c].items():
                sim.cores[c].tensor(name)[:] = val
        sim.simulate()
        results = [{"out": np.asarray(sim.cores[c].tensor("out"))}
                   for c in range(NCORES)]
    else:
        from concourse.bass_utils import run_bass_kernel_spmd
        try:
            try:
                res = run_bass_kernel_spmd(nc, in_maps,
                                           core_ids=list(range(NCORES)),
                                           trace=_trace)
            except ModuleNotFoundError:
                res = run_bass_kernel_spmd(nc, in_maps,
                                           core_ids=list(range(NCORES)),
                                           trace=False)
            _CACHE["last"] = res
            results = res.results
        except Exception:
            # hardware path unavailable: fall back to multi-core simulator
            from concourse import bass_interp
            sim = bass_interp.MultiCoreSim(nc, NCORES)
            for c in range(NCORES):
                for name, val in in_maps[c].items():
                    sim.cores[c].tensor(name)[:] = val
            sim.simulate()
            results = [{"out": np.asarray(sim.cores[c].tensor("out"))}
                       for c in range(NCORES)]
    out = np.empty((B, CO, H, W), np.float32)
    for c in range(NCORES):
        b, rb = c // 4, c % 4
        o = results[c]["out"]  # [2, 128, NPOS]
        ot = o.reshape(CO, 6, 384)[:, :, _LUT]       # [CO, 6T, col]
        out[b, :, rb * RB:(rb + 1) * RB, :] = ot.reshape(CO, RB, W)
    return out


# revision 23
# speedup vs baseline: 3.7513x; 1.0416x over previous
"""DCNv2 (deformable conv + BN + ReLU) Trainium2 Bass kernel, 8-core SPMD.

v2: fully pipelined per T-tile (4 output rows each). Core c owns sample
b=c//4, output rows [24*(c%4), 24*(c%4)+24).

Position relabeling: within a T-tile, conv column col = t*96+w is assigned
pipeline position l = q*128 + pp*16 + r where col = r*24 + q*8 + pp.
This makes the gather-index repack DMA contiguous in 48B runs:
  idxG[r, k*24 + c] = idx16[k, r*24 + c]   (c = col%24)
and the gather consumes idxG[16, 216] in n = s*16+r order with
n = ((k%3)*3+q)*128 + (pp*16+r), exactly the corner-matmul layout.

Per T (program order; pools give cross-T overlap):
  conv(T) on PE -> coeffs(T) on DVE/ACT (conv layout [27, 384]) ->
  a-transpose on PE (3x [36,128]->[128,36]) -> idx DMA roundtrip ->
  3x dma_gather -> dg = ident*a (1 broadcast DVE op per q) ->
  corner matmuls (216) -> main GEMM (36) with BN sums via accum_out.
Tail: AllReduce of BN stats, scale/shift/ReLU, chunked stores.
"""

import numpy as np
import ml_dtypes

BF16 = ml_dtypes.bfloat16
B, CI, CO, H, W = 2, 256, 256, 96, 96
NCORES = 8
RB = 24                      # output rows per core
NPOS = RB * W                # 2304 positions per core
PADG = 8                     # gather-table pad on each side
GRID = H + 2 * PADG          # 112
NROWS = GRID * GRID          # 12544 table rows
NTOT = float(B * H * W)      # BN count
EPS = 1e-5
MAGIC = 8388608.0            # 2^23 for round-to-floor trick

KY9 = np.repeat(np.arange(3), 3).astype(np.float32)
KX9 = np.tile(np.arange(3), 3).astype(np.float32)

_CACHE = {}


def _build_program():
    import concourse.bass as bass
    from concourse import bacc, tile, mybir

    ds = bass.ds
    f32 = mybir.dt.float32
    bf16 = mybir.dt.bfloat16
    i16 = mybir.dt.int16
    Alu = mybir.AluOpType
    Act = mybir.ActivationFunctionType

    nc = bacc.Bacc("TRN2", target_bir_lowering=False, debug=False,
                   num_devices=NCORES, dynamic_dma_scratch_size=32768)

    tab_d = nc.dram_tensor("tab", [NROWS, 1024], bf16, kind="ExternalInput")
    slab_d = nc.dram_tensor("slab", [128, 2, RB + 2, W + 2], bf16,
                            kind="ExternalInput")
    woff_d = nc.dram_tensor("woff", [128, 2, 9, 96], bf16,
                            kind="ExternalInput")
    pypx_d = nc.dram_tensor("pypx", [96, 6, 384], f32, kind="ExternalInput")
    wdcn_d = nc.dram_tensor("wdcn", [128, 18, 2, 128], bf16,
                            kind="ExternalInput")
    ident_d = nc.dram_tensor("ident", [128, 128], bf16, kind="ExternalInput")
    identf_d = nc.dram_tensor("identf", [128, 128], f32, kind="ExternalInput")
    gb_d = nc.dram_tensor("gb", [128, 2, 3], f32, kind="ExternalInput")
    out_d = nc.dram_tensor("out", [2, 128, NPOS], f32, kind="ExternalOutput")

    with tile.TileContext(nc) as tc:
        with (
            tc.tile_pool(name="cst", bufs=1) as cst,
            tc.tile_pool(name="sb", bufs=1) as sb,
            tc.tile_pool(name="cf", bufs=2) as cf,
            tc.tile_pool(name="gpool", bufs=3) as gpool,
            tc.tile_pool(name="apool", bufs=2) as apool,
            tc.tile_pool(name="dpool", bufs=6) as dpool,
            tc.tile_pool(name="spool", bufs=2) as spool,
            tc.tile_pool(name="opool", bufs=3) as opool,
            tc.tile_pool(name="ps_om", bufs=1, space="PSUM") as ps_om,
            tc.tile_pool(name="ps_t", bufs=1, space="PSUM") as ps_t,
            tc.tile_pool(name="ps_s", bufs=2, space="PSUM") as ps_s,
            tc.tile_pool(name="ps_o", bufs=2, space="PSUM") as ps_o,
            tc.tile_pool(name="dram", bufs=1, space="DRAM") as dram,
        ):
            # ---------- PE warm-up: ramp p-state during input loads ----
            wident = cst.tile([128, 128], bf16)
            nc.vector.memset(wident[:], 0)
            wps = ps_om.tile([96, 384], f32, tag="pom")
            for _ in range(40):
                nc.tensor.matmul(wps[:, 0:128], wident[:, 0:96],
                                 wident[:])

            # ---------- persistent tiles ----------
            slab = cst.tile([128, 2, RB + 2, W + 2], bf16)
            nc.sync.dma_start(slab[:], slab_d[:])
            woff = cst.tile([128, 2, 9, 96], bf16)
            nc.sync.dma_start(woff[:], woff_d[:])
            pypx = cst.tile([96, 6, 384], f32)
            nc.sync.dma_start(pypx[:], pypx_d[:])
            wdcn = cst.tile([128, 18, 2, 128], bf16)
            nc.sync.dma_start(wdcn[:], wdcn_d[:])
            ident = cst.tile([128, 128], bf16)
            nc.sync.dma_start(ident[:], ident_d[:])
            identf = cst.tile([128, 128], f32)
            nc.sync.dma_start(identf[:], identf_d[:])
            gb = cst.tile([128, 2, 3], f32)
            nc.sync.dma_start(gb[:], gb_d[:])

            idxG = sb.tile([128, 2, 216], i16)
            nc.vector.memset(idxG[:], 0)
            d4 = dram.tile([2, 3456], i16)
            out_sb = sb.tile([128, 2, NPOS], bf16)
            SU = sb.tile([128, 2, 18], f32)  # per-(T,q) BN sums
            SQ = sb.tile([128, 2, 6], f32)   # per-T BN sum-of-squares

            ident_b = ident[:].rearrange("p (one n) -> p one n", one=1) \
                .broadcast_to([128, 36, 128])

            def conv(T):
                pom = ps_om.tile([96, 384], f32, tag="pom")
                first = True
                for ct in range(2):
                    for k in range(9):
                        ky, kx = int(KY9[k]), int(KX9[k])
                        rhs = slab[:, ct, T * 4 + ky:T * 4 + ky + 4,
                                   kx:kx + 96]
                        nc.tensor.matmul(pom[:], woff[:, ct, k, :], rhs,
                                         start=first,
                                         stop=(ct == 1 and k == 8))
                        first = False
                return pom

            def idx_coeffs(T, pom):
                # coefficients in conv layout [<=36 part, 384 col]
                opp = cf.tile([96, 384], f32, tag="opp")
                nc.vector.tensor_tensor(opp[:], pom[:], pypx[:, T], Alu.add)
                msk = cf.tile([9, 384], f32, tag="msk")
                nc.scalar.activation(msk[:], opp[64:73], Act.Sigmoid)
                iyx = cf.tile([64, 384], f32, tag="iyx")
                # floor via round(x - 0.5); exact-int x floors one low
                # (harmless by bilinear continuity). y rows 0:9, x rows
                # 32:41; in-between rows are well-defined junk.
                nc.vector.tensor_scalar(iyx[:], opp[0:64], MAGIC - 0.5,
                                        -MAGIC, Alu.add, Alu.add)
                fyx = cf.tile([64, 384], f32, tag="fyx")
                nc.vector.tensor_tensor(fyx[:], opp[0:64], iyx[:],
                                        Alu.subtract)
                nc.vector.tensor_scalar(iyx[:], iyx[:], 8.0, 118.0, Alu.max,
                                        Alu.min)
                ix9 = cf.tile([9, 384], f32, tag="ix9")
                nc.vector.tensor_copy(ix9[:], iyx[32:41])
                idxf = cf.tile([9, 384], f32, tag="idxf")
                nc.vector.tensor_scalar(idxf[:], iyx[0:9], float(GRID),
                                        -904.0, Alu.mult, Alu.add)
                nc.vector.tensor_tensor(idxf[:], idxf[:], ix9[:], Alu.add)
                idx16 = cf.tile([9, 384], i16, tag="idx16")
                nc.vector.tensor_copy(idx16[:], idxf[:])

                # idx repack via DRAM (contiguous 48B runs), then gathers
                slot = T % 2
                nc.sync.dma_start(
                    d4[slot].rearrange("(r k c) -> k r c", r=16, k=9),
                    idx16[:].rearrange("k (r c) -> k r c", r=16))
                nc.sync.dma_start(idxG[0:16, slot, :],
                                  d4[slot].rearrange("(r s) -> r s", r=16))
                gt = []
                for kc in range(3):
                    g = gpool.tile([128, 9, 1024], bf16, tag="g")
                    nc.gpsimd.dma_gather(
                        g[:], tab_d[:], idxG[:, slot, kc * 72:(kc + 1) * 72],
                        num_idxs=1152, num_idxs_reg=1152, elem_size=1024)
                    gt.append(g)
                return msk, fyx, gt

            def a_coeffs(T, fyx):
                # tensor_tensor operands must share a base partition, so
                # copy the x rows (base 32) down to base-0 tiles first
                wyx0 = cf.tile([64, 384], f32, tag="wyx0")
                nc.gpsimd.tensor_scalar(wyx0[:], fyx[:], -1.0, 1.0, Alu.mult,
                                        Alu.add)
                wx9 = cf.tile([9, 384], f32, tag="wx9")
                nc.gpsimd.tensor_copy(wx9[:], wyx0[32:41])
                fx9 = cf.tile([9, 384], f32, tag="fx9")
                nc.gpsimd.tensor_copy(fx9[:], fyx[32:41])
                aFj = cf.tile([9, 4, 384], f32, tag="aFj")
                nc.gpsimd.tensor_tensor(aFj[:, 0, :], wyx0[0:9], wx9[:],
                                        Alu.mult)
                nc.gpsimd.tensor_tensor(aFj[:, 1, :], wyx0[0:9], fx9[:],
                                        Alu.mult)
                nc.gpsimd.tensor_tensor(aFj[:, 2, :], fyx[0:9], wx9[:],
                                        Alu.mult)
                nc.gpsimd.tensor_tensor(aFj[:, 3, :], fyx[0:9], fx9[:],
                                        Alu.mult)
                return aFj

            def a_transpose(T, aFj, msk):
                # PE transposes per (q, j): [9, (pp,r)=128] -> [128, 9]
                # packed at cols j*9+k; mask -> cols 36:45
                a_ps = ps_t.tile([128, 3, 48], f32, tag="aps")
                aF_v = aFj[:].rearrange("k j (r q2 pp) -> k j q2 pp r",
                                        r=16, q2=3)
                m_v = msk[:].rearrange("k (r q2 pp) -> k q2 pp r",
                                       r=16, q2=3)
                for q in range(3):
                    for j in range(4):
                        nc.tensor.matmul(a_ps[:, q, j * 9:(j + 1) * 9],
                                         aF_v[:, j, q],
                                         identf[0:9, 0:9], is_transpose=True)
                    nc.tensor.matmul(a_ps[:, q, 36:45], m_v[:, q],
                                     identf[0:9, 0:9], is_transpose=True)
                a_pos = apool.tile([128, 3, 48], f32, tag="apos")
                nc.vector.tensor_copy(a_pos[:, :, 0:45], a_ps[:, :, 0:45])
                return a_pos

            def dg_one(T, t, q, a_pos):
                # 12 diags for (third t, q-block): k in {3t..3t+2} x 4 corners
                dgs = dpool.tile([128, 12, 128], bf16, tag="dg")
                for kk in range(3):
                    k = t * 3 + kk
                    for j in range(4):
                        nc.vector.tensor_scalar(
                            dgs[:, kk * 4 + j, :], ident[:],
                            a_pos[:, q, j * 9 + k:j * 9 + k + 1],
                            a_pos[:, q, 36 + k:37 + k],
                            Alu.mult, Alu.mult)
                return dgs

            def corner_tq(T, t, q, g, dgs, s_sb):
                # third t only reads gather tile t
                pss = ps_s.tile([128, 6, 128], f32, tag="pss")
                for chl in range(6):
                    k, cfh = t * 3 + chl // 2, chl % 2
                    slot9 = (k % 3) * 3 + q
                    for j in range(4):
                        lhsT = g[:, slot9, j * 256 + cfh * 128:
                                 j * 256 + cfh * 128 + 128]
                        nc.tensor.matmul(pss[:, chl, :], lhsT,
                                         dgs[:, (chl // 2) * 4 + j, :],
                                         start=(j == 0), stop=(j == 3))
                nc.scalar.copy(s_sb[:, t * 6:t * 6 + 6,
                                    q * 128:(q + 1) * 128], pss[:])

            def gemm_q(T, q, s_sb):
                po = ps_o.tile([128, 2, 128], f32, tag="po")
                for o2 in range(2):
                    for ch in range(18):
                        nc.tensor.matmul(po[:, o2, :], wdcn[:, ch, o2, :],
                                         s_sb[:, ch, q * 128:(q + 1) * 128],
                                         start=(ch == 0), stop=(ch == 17))
                for o2 in range(2):
                    osl = out_sb[:, o2, T * 384 + q * 128:
                                 T * 384 + (q + 1) * 128]
                    nc.scalar.activation(osl, po[:, o2, :], Act.Identity,
                                         bias=gb[:, o2, 2:3],
                                         accum_out=SU[:, o2,
                                                      T * 3 + q:T * 3 + q + 1])

            def square(T):
                for o2 in range(2):
                    scrap = sb.tile([128, 384], bf16, tag="scrap")
                    nc.scalar.activation(scrap[:],
                                         out_sb[:, o2,
                                                T * 384:(T + 1) * 384],
                                         Act.Square,
                                         accum_out=SQ[:, o2, T:T + 1])

            # ---------- software-pipelined main loop ----------
            # corner loops are third-major: third t consumes only gather
            # tile t, so compute starts as soon as the first gather lands
            pom = conv(0)
            msk, fyx, gt = idx_coeffs(0, pom)
            aF = a_coeffs(0, fyx)
            a_pos = a_transpose(0, aF, msk)
            junk = sb.tile([1, 2], f32)
            for T in range(6):
                if T == 5:
                    # preload sqrt act-table after the last Sigmoid (the
                    # Square dep pins it late; Relu/Copy/Square are in the
                    # sqrt set too, so the tail needs no further switch)
                    nc.scalar.activation(junk[:, 0:1], SU[0:1, 1, 14:15],
                                         Act.Square)
                    nc.scalar.sqrt(junk[:, 1:2], junk[:, 0:1])
                s_sb = spool.tile([128, 18, 384], bf16, tag="s")
                d0 = [dg_one(T, 0, q, a_pos) for q in range(3)]
                d1 = [dg_one(T, 1, q, a_pos) for q in range(3)]
                for q in range(3):
                    corner_tq(T, 0, q, gt[0], d0[q], s_sb)
                if T < 5:
                    pom = conv(T + 1)
                    msk, fyx, gt_n = idx_coeffs(T + 1, pom)
                for q in range(3):
                    corner_tq(T, 1, q, gt[1], d1[q], s_sb)
                if T < 5:
                    aF = a_coeffs(T + 1, fyx)
                d2 = [dg_one(T, 2, 0, a_pos), dg_one(T, 2, 1, a_pos), None]
                corner_tq(T, 2, 0, gt[2], d2[0], s_sb)
                if T < 5:
                    a_posn = a_transpose(T + 1, aF, msk)
                gemm_q(T, 0, s_sb)
                d2[2] = dg_one(T, 2, 2, a_pos)
                corner_tq(T, 2, 1, gt[2], d2[1], s_sb)
                gemm_q(T, 1, s_sb)
                corner_tq(T, 2, 2, gt[2], d2[2], s_sb)
                gemm_q(T, 2, s_sb)
                square(T)
                if T < 5:
                    gt = gt_n
                    a_pos = a_posn

            # ---------- BN stats + allreduce + finish ----------
            part = sb.tile([128, 4], f32)
            for o2 in range(2):
                nc.vector.tensor_reduce(part[:, 2 * o2:2 * o2 + 1],
                                        SU[:, o2, :],
                                        mybir.AxisListType.X, Alu.add)
                nc.vector.tensor_reduce(part[:, 2 * o2 + 1:2 * o2 + 2],
                                        SQ[:, o2, :],
                                        mybir.AxisListType.X, Alu.add)
            bin_d = dram.tile([128, 4], f32)
            bout_d = dram.tile([128, 4], f32, addr_space="Shared")
            import os as _os
            nc.sync.dma_start(bin_d[:], part[:])
            if _os.environ.get("NOCC", "0") == "1":
                nc.gpsimd.dma_start(bout_d[:], bin_d[:])
            else:
                nc.gpsimd.collective_compute(
                    "AllReduce", mybir.AluOpType.add,
                    replica_groups=[list(range(NCORES))],
                    ins=[bin_d[:].opt()], outs=[bout_d[:].opt()])
            stats = sb.tile([128, 4], f32)
            nc.sync.dma_start(stats[:], bout_d[:])
            tmp = sb.tile([128, 8], f32)
            for o2 in range(2):
                mean = tmp[:, 4 * o2 + 0:4 * o2 + 1]
                var = tmp[:, 4 * o2 + 1:4 * o2 + 2]
                s_ = tmp[:, 4 * o2 + 2:4 * o2 + 3]
                t_ = tmp[:, 4 * o2 + 3:4 * o2 + 4]
                nc.vector.tensor_scalar_mul(mean, stats[:, 2 * o2:2 * o2 + 1],
                                            1.0 / NTOT)
                nc.vector.tensor_scalar_mul(var,
                                            stats[:, 2 * o2 + 1:2 * o2 + 2],
                                            1.0 / NTOT)
                nc.vector.tensor_tensor(s_, mean, mean, Alu.mult)
                nc.vector.tensor_tensor(var, var, s_, Alu.subtract)
                nc.vector.tensor_scalar_add(var, var, EPS)
                nc.scalar.sqrt(s_, var)
                nc.vector.reciprocal(s_, s_)
                nc.vector.tensor_tensor(s_, s_, gb[:, o2, 0:1], Alu.mult)
                nc.vector.tensor_tensor(t_, mean, s_, Alu.mult)
                nc.vector.tensor_scalar_mul(t_, t_, -1.0)
                nc.vector.tensor_tensor(t_, t_, gb[:, o2, 1:2], Alu.add)
            for o2 in range(2):
                s_ = tmp[:, 4 * o2 + 2:4 * o2 + 3]
                t_ = tmp[:, 4 * o2 + 3:4 * o2 + 4]
                for hh in range(4):
                    outf = opool.tile([128, 576], f32, tag="outf")
                    nc.scalar.activation(
                        outf[:], out_sb[:, o2, hh * 576:(hh + 1) * 576],
                        Act.Relu, bias=t_, scale=s_)
                    nc.sync.dma_start(out_d[o2, :, hh * 576:(hh + 1) * 576],
                                      outf[:])

    nc.compile()
    return nc


# position permutation: l = q*128 + pp*16 + r for col = r*24 + q*8 + pp
_COL = np.arange(384)
_LUT = (_COL % 24 // 8) * 128 + (_COL % 8) * 16 + _COL // 24  # col -> l


def _prep_inputs(x, w_off, b_off, w_dcn, b_dcn, gamma, beta):
    """Build the 8 per-core input maps (host-side sharding/layout only)."""
    x = np.asarray(x, np.float32)
    w_off = np.asarray(w_off, np.float32)
    b_off = np.asarray(b_off, np.float32)
    w_dcn = np.asarray(w_dcn, np.float32)
    b_dcn = np.asarray(b_dcn, np.float32)
    gamma = np.asarray(gamma, np.float32)
    beta = np.asarray(beta, np.float32)

    # 4-corner gather tables per sample
    P = PADG
    xp = np.zeros((B, CI, GRID + 1, GRID + 1), np.float32)
    xp[:, :, P:P + H, P:P + W] = x
    xp = xp.astype(BF16)
    tabs = []
    for b in range(B):
        t = np.empty((GRID, GRID, 4, CI), BF16)
        for j, (dy2, dx2) in enumerate([(0, 0), (0, 1), (1, 0), (1, 1)]):
            t[:, :, j, :] = np.moveaxis(
                xp[b, :, dy2:dy2 + GRID, dx2:dx2 + GRID], 0, -1)
        tabs.append(np.ascontiguousarray(t.reshape(NROWS, 1024)))

    # conv slab (1-pixel zero pad) per sample, bf16, [128, ct, 26, 98]
    xs = np.zeros((B, CI, H + 2, W + 2), np.float32)
    xs[:, :, 1:H + 1, 1:W + 1] = x
    xs = xs.astype(BF16)

    # offset-conv weights, output channels permuted to [dy*9, dx*9, m*9]
    perm = np.concatenate([np.arange(0, 17, 2), np.arange(1, 18, 2),
                           np.arange(18, 27)])
    wofp = w_off[perm]            # [27, CI, 3, 3]
    boffp = b_off[perm]
    w27 = np.ascontiguousarray(
        wofp.reshape(27, 2, 128, 3, 3).transpose(2, 1, 3, 4, 0)
        .reshape(128, 2, 9, 27)).astype(BF16)
    # out channels embedded at partition groups 0:9 (y), 32:41 (x),
    # 64:73 (m) so engine slices start at multiples of 32
    woff_h = np.zeros((128, 2, 9, 96), BF16)
    woff_h[:, :, :, 0:9] = w27[:, :, :, 0:9]
    woff_h[:, :, :, 32:41] = w27[:, :, :, 9:18]
    woff_h[:, :, :, 64:73] = w27[:, :, :, 18:27]

    # wdcn lhsT chunks: [p, ch=(k*2+cf), o2, oc] = w_dcn[o2*128+oc, cf*128+p, k]
    wd = w_dcn.reshape(CO, CI, 9)
    wdcn_h = np.ascontiguousarray(
        wd.reshape(2, 128, 2, 128, 9).transpose(3, 4, 2, 0, 1)
        .reshape(128, 9, 2, 2, 128).transpose(0, 1, 2, 3, 4)
        .reshape(128, 18, 2, 128)).astype(BF16)

    ident_h = np.eye(128, dtype=BF16)
    identf_h = np.eye(128, dtype=np.float32)
    gb_h = np.zeros((128, 2, 3), np.float32)
    for o2 in range(2):
        gb_h[:, o2, 0] = gamma[o2 * 128:(o2 + 1) * 128]
        gb_h[:, o2, 1] = beta[o2 * 128:(o2 + 1) * 128]
        gb_h[:, o2, 2] = b_dcn[o2 * 128:(o2 + 1) * 128]

    tt = np.arange(4, dtype=np.float32)   # row within T-tile
    ww = np.arange(96, dtype=np.float32)
    in_maps = []
    for c in range(NCORES):
        b, rb = c // 4, c % 4
        slab_h = np.ascontiguousarray(
            xs[b].reshape(2, 128, H + 2, W + 2)
            .transpose(1, 0, 2, 3)[:, :, rb * RB:rb * RB + RB + 2, :])
        # pypx [96, 6T, 384col]: +16 (grid offset) folded into y/x rows;
        # rows 0:9 = y base, 32:41 = x base, 64:73 = mask bias
        pypx_h = np.zeros((96, 6, 384), np.float32)
        for T in range(6):
            py = np.broadcast_to(
                rb * RB + T * 4 - 1.0 + 16.0 + tt[None, :, None]
                + KY9[:, None, None] + boffp[0:9, None, None], (9, 4, 96))
            px = (ww[None, None, :] - 1.0 + 16.0
                  + KX9[:, None, None] + boffp[9:18, None, None])
            px = np.broadcast_to(px, (9, 4, 96))
            pypx_h[0:9, T] = py.reshape(9, 384)
            pypx_h[32:41, T] = px.reshape(9, 384)
            pypx_h[64:73, T] = boffp[18:27, None]
        in_maps.append({
            "tab": tabs[b], "slab": slab_h, "woff": woff_h,
            "pypx": pypx_h, "wdcn": wdcn_h, "ident": ident_h,
            "identf": identf_h, "gb": gb_h,
        })
    return in_maps


def kernel(x, w_off, b_off, w_dcn, b_dcn, gamma, beta, _trace=False):
    import os
    if "nc" not in _CACHE:
        _CACHE["nc"] = _build_program()
    nc = _CACHE["nc"]
    in_maps = _prep_inputs(x, w_off, b_off, w_dcn, b_dcn, gamma, beta)
    results = None
    if os.environ.get("FORCE_SIM", "0") == "1":
        from concourse import bass_interp
        sim = bass_interp.MultiCoreSim(nc, NCORES)
        for c in range(NCORES):
            for name, val in in_maps[c].items():
                sim.cores[c].tensor(name)[:] = val
        sim.simulate()
        results = [{"out": np.asarray(sim.cores[c].tensor("out"))}
                   for c in range(NCORES)]
    else:
        from concourse.bass_utils import run_bass_kernel_spmd
        try:
            try:
                res = run_bass_kernel_spmd(nc, in_maps,
                                           core_ids=list(range(NCORES)),
                                           trace=_trace)
            except ModuleNotFoundError:
                res = run_bass_kernel_spmd(nc, in_maps,
                                           core_ids=list(range(NCORES)),
                                           trace=False)
            _CACHE["last"] = res
            results = res.results
        except Exception:
            # hardware path unavailable: fall back to multi-core simulator
            from concourse import bass_interp
            sim = bass_interp.MultiCoreSim(nc, NCORES)
            for c in range(NCORES):
                for name, val in in_maps[c].items():
                    sim.cores[c].tensor(name)[:] = val
            sim.simulate()
            results = [{"out": np.asarray(sim.cores[c].tensor("out"))}
                       for c in range(NCORES)]
    out = np.empty((B, CO, H, W), np.float32)
    for c in range(NCORES):
        b, rb = c // 4, c % 4
        o = results[c]["out"]  # [2, 128, NPOS]
        ot = o.reshape(CO, 6, 384)[:, :, _LUT]       # [CO, 6T, col]
        out[b, :, rb * RB:(rb + 1) * RB, :] = ot.reshape(CO, RB, W)
    return out
